# revision 11
# baseline (speedup 1.0000x reference)
"""GATv2 attention-pool kernel for 8 Trainium2 NeuronCores.

Algorithm
---------
Reference computes, per edge e with target node t(e):
    feats = q + k                                   [E, 64]
    logits[e,h] = sum_c feats[e,h*8+c] * A[c,h]     [E, 8]
    attn = segment_softmax(logits, targets)         [E, 8]
    out[n] = relu(segment_sum(q * attn))            [N, 64]

Because logits are O(10), exp() never overflows fp32, so the segment-max
shift is unnecessary and softmax folds into two segment-SUMS that share
one pass:
    denom[n,h]  = sum_{e->n} exp(logits[e,h])
    pooled[n,:] = sum_{e->n} q[e,:] * exp(logits[e,h])
    out[n]      = relu(pooled[n]) / denom[n]        (denom > 0 always)

Distribution: edges are partitioned by target node (host-side sort), 100000
nodes split into 8 contiguous shards of 12500 -> all segment reductions are
core-local, no collectives.  Each shard is cut into 196 windows of 64 nodes;
a window's edges are padded to T_w * 128 slots (T_w identical across cores so
one SPMD program serves all 8 cores).  Per 128-edge subtile the device builds
a one-hot selector S[e, n_rel] = (rel[e] == n_rel) and accumulates
    psum[64, 72] += S^T @ [q*ex | ex]
on the PE across the window's subtiles, then divides / relus once per node.

Host work is index metadata + data layout only (argsort of targets, gather
of q/k rows into the sorted slot order); all floating-point math runs on
device.
"""

import os
import sys

import numpy as np

N_NODES = 100000
N_EDGES = 1600000
H = 8
C = 8
HC = H * C
N_CORES = 8
NODES_PER_CORE = N_NODES // N_CORES
WIN_NODES = 64
SUB = 128


def _ensure_imports():
    try:
        import concourse.bass  # noqa: F401
    except ImportError:
        for p in ("/opt/trn_rl_repo", "/root/.axon_site/_ro/trn_rl_repo"):
            if os.path.isdir(p) and p not in sys.path:
                sys.path.insert(0, p)


def preprocess(targets, n_nodes, n_cores, win_nodes):
    """Sort edges by target; compute per-window slot layout shared by cores.

    Returns (perms [n_cores, n_slots] edge ids, rels [n_cores, n_slots] f32
    rel-node-or--1, T [n_win] subtiles per window, n_slots).
    """
    nodes_per_core = n_nodes // n_cores
    wins_per_core = (nodes_per_core + win_nodes - 1) // win_nodes
    order = np.argsort(targets, kind="stable")
    tsorted = targets[order]

    bounds = np.empty(n_cores * wins_per_core + 1, dtype=np.int64)
    i = 0
    for c in range(n_cores):
        for w in range(wins_per_core):
            bounds[i] = c * nodes_per_core + w * win_nodes
            i += 1
    bounds[-1] = n_nodes
    starts = np.searchsorted(tsorted, bounds[:-1], side="left")
    ends = np.concatenate([starts[1:], [len(tsorted)]])
    counts = (ends - starts).reshape(n_cores, wins_per_core)

    T = np.maximum(1, (counts.max(axis=0) + SUB - 1) // SUB).astype(np.int64)
    slots_per_win = T * SUB
    win_slot_base = np.concatenate([[0], np.cumsum(slots_per_win)])
    n_slots = int(win_slot_base[-1])

    perms = np.zeros((n_cores, n_slots), dtype=np.int64)
    rels = np.full((n_cores, n_slots), -1.0, dtype=np.float32)
    for c in range(n_cores):
        for w in range(wins_per_core):
            j = c * wins_per_core + w
            e0, e1 = starts[j], ends[j]
            sb = win_slot_base[w]
            cnt = e1 - e0
            perms[c, sb:sb + cnt] = order[e0:e1]
            rels[c, sb:sb + cnt] = (
                tsorted[e0:e1] - (c * nodes_per_core + w * win_nodes)
            ).astype(np.float32)
    return perms, rels, T, n_slots


def build_nc(T, n_slots, out_rows):
    """Build the single SPMD Bass program for one core's shard."""
    _ensure_imports()
    import concourse.bacc as bacc
    import concourse.mybir as mybir
    import concourse.tile as tile

    f32 = mybir.dt.float32
    Tmax = int(max(T))
    n_win = len(T)

    # process windows in pairs: one set of wide tiles per group amortizes
    # DVE per-op overhead and doubles DMA transfer sizes
    groups = []
    wb = 0
    w = 0
    while w < n_win:
        pair = [(w, int(T[w]), wb)]
        wb += int(T[w]) * SUB
        w += 1
        if w < n_win:
            pair.append((w, int(T[w]), wb))
            wb += int(T[w]) * SUB
            w += 1
        groups.append(pair)
    Tgmax = max(sum(t for _, t, _ in g) for g in groups)

    nc = bacc.Bacc("TRN2", num_devices=N_CORES)
    qk = nc.declare_dram_parameter("qk", [n_slots, 2 * HC], f32, False)
    rel = nc.declare_dram_parameter("rel", [n_slots], f32, False)
    wrow = nc.declare_dram_parameter("wrow", [128, Tgmax * HC], f32, False)
    iota = nc.declare_dram_parameter("iota", [128, WIN_NODES], f32, False)
    out = nc.declare_dram_parameter("out", [out_rows, HC], f32, isOutput=True)

    AX = mybir.AxisListType
    OP = mybir.AluOpType
    AF = mybir.ActivationFunctionType
    MW = 2 * HC  # qk row width

    with tile.TileContext(nc) as tc:
        with (
            tc.tile_pool(name="const", bufs=1) as cpool,
            tc.tile_pool(name="qk", bufs=4) as qkpool,
            tc.tile_pool(name="mid", bufs=3) as midpool,
            tc.tile_pool(name="mm", bufs=4) as mmpool,
            tc.tile_pool(name="fin", bufs=3) as finpool,
            tc.tile_pool(name="psum", bufs=6, space="PSUM") as ppool,
        ):
            w_t = cpool.tile([128, Tgmax * HC], f32)
            nc.sync.dma_start(out=w_t[:], in_=wrow[:])
            io_t = cpool.tile([128, WIN_NODES], f32)
            nc.sync.dma_start(out=io_t[:], in_=iota[:])

            for pair in groups:
                Tg = sum(t for _, t, _ in pair)
                fd = Tg * HC

                qk_t = qkpool.tile([128, Tg * MW], f32, tag="qk")
                r_t = qkpool.tile([128, Tg], f32, tag="r")
                off = 0
                for _, Tw, wbase in pair:
                    nsl = Tw * SUB
                    nc.sync.dma_start(
                        out=qk_t[:, off * MW:(off + Tw) * MW],
                        in_=qk[wbase:wbase + nsl, :].rearrange(
                            "(p t) c -> p (t c)", p=128),
                    )
                    nc.sync.dma_start(
                        out=r_t[:, off:off + Tw],
                        in_=rel[wbase:wbase + nsl].rearrange(
                            "(p t) -> p t", p=128),
                    )
                    off += Tw

                qk3 = qk_t[:].rearrange("p (t c) -> p t c", c=MW)

                # feats = q + k   (GpSimd)
                f_t = midpool.tile([128, fd], f32, tag="f")
                nc.gpsimd.tensor_add(
                    f_t[:], qk3[:, :, 0:HC], qk3[:, :, HC:MW]
                )

                # S one-hot: (rel == iota)
                s_t = mmpool.tile([128, Tg, WIN_NODES], f32, tag="S")
                nc.vector.tensor_tensor(
                    out=s_t[:],
                    in0=r_t[:, :, None].to_broadcast([128, Tg, WIN_NODES]),
                    in1=io_t[:, None, :].to_broadcast([128, Tg, WIN_NODES]),
                    op=OP.is_equal,
                )

                # wf = feats * Wrow ; logits = sum_c wf (split DVE/GpSimd)
                wf_t = midpool.tile([128, fd], f32, tag="wf")
                fd2 = (fd // 2) & ~63
                if fd2:
                    nc.gpsimd.tensor_mul(
                        wf_t[:, :fd2], f_t[:, :fd2], w_t[:, :fd2])
                nc.vector.tensor_mul(
                    wf_t[:, fd2:fd], f_t[:, fd2:fd], w_t[:, fd2:fd])
                lg_t = midpool.tile([128, Tg * H], f32, tag="lg")
                nc.vector.tensor_reduce(
                    out=lg_t[:],
                    in_=wf_t[:].rearrange("p (t h c) -> p (t h) c", h=H, c=C),
                    axis=AX.X,
                    op=OP.add,
                )

                # M = [q*ex | ex]
                m_t = mmpool.tile([128, Tg, H * C + H], f32, tag="M")
                nc.scalar.activation(
                    out=m_t[:, :, HC:HC + H],
                    in_=lg_t[:].rearrange("p (t h) -> p t h", h=H),
                    func=AF.Exp,
                )
                nc.vector.tensor_mul(
                    m_t[:, :, 0:HC].rearrange("p t (h c) -> p t h c", h=H),
                    qk3[:, :, 0:HC].rearrange("p t (h c) -> p t h c", h=H),
                    m_t[:, :, HC:HC + H, None].to_broadcast([128, Tg, H, C]),
                )

                # psum[64, 72*win] += S_g^T @ M_g  per window in the pair
                p_t = ppool.tile([WIN_NODES, len(pair) * (HC + H)], f32)
                off = 0
                for wi, (_, Tw, _) in enumerate(pair):
                    pcols = slice(wi * (HC + H), wi * (HC + H) + HC + H)
                    for g in range(Tw):
                        nc.tensor.matmul(
                            p_t[:, pcols],
                            lhsT=s_t[:, off + g, :],
                            rhs=m_t[:, off + g, :],
                            start=(g == 0),
                            stop=(g == Tw - 1),
                        )
                    off += Tw

                # out = relu(pooled) / denom, batched over the pair
                nw = len(pair)
                p3 = p_t[:].rearrange("p (w j) -> p w j", j=HC + H)
                relu_t = finpool.tile([WIN_NODES, nw, HC], f32, tag="relu")
                nc.scalar.activation(relu_t[:], p3[:, :, 0:HC], func=AF.Relu)
                rc_t = finpool.tile([WIN_NODES, nw, H], f32, tag="rc")
                nc.vector.reciprocal(rc_t[:], p3[:, :, HC:HC + H])
                o_t = finpool.tile([WIN_NODES, nw, HC], f32, tag="o")
                nc.gpsimd.tensor_mul(
                    o_t[:].rearrange("p w (h c) -> p w h c", h=H),
                    relu_t[:].rearrange("p w (h c) -> p w h c", h=H),
                    rc_t[:, :, :, None].to_broadcast(
                        [WIN_NODES, nw, H, C]),
                )
                w0 = pair[0][0]
                nc.sync.dma_start(
                    out=out[w0 * WIN_NODES:(w0 + nw) * WIN_NODES, :]
                    .rearrange("(w p) c -> p w c", w=nw),
                    in_=o_t[:],
                )

    nc.finalize()
    return nc


def _host_arrays(query, key, attn_kernel, targets):
    perms, rels, T, n_slots = preprocess(
        targets, N_NODES, N_CORES, WIN_NODES
    )
    # widest 2-window group determines the resident W tile width
    Tg = [int(T[i]) + (int(T[i + 1]) if i + 1 < len(T) else 0)
          for i in range(0, len(T), 2)]
    Tgmax = max(Tg)
    wrow_1 = np.ascontiguousarray(attn_kernel.T).reshape(-1)  # [h*8+c] = A[c,h]
    wrow = np.tile(wrow_1, (128, Tgmax)).astype(np.float32)
    iota = np.tile(
        np.arange(WIN_NODES, dtype=np.float32), (128, 1)
    )
    in_maps = []
    for c in range(N_CORES):
        qkc = np.empty((n_slots, 2 * HC), dtype=np.float32)
        qkc[:, :HC] = query[perms[c]]
        qkc[:, HC:] = key[perms[c]]
        in_maps.append({
            "qk": qkc,
            "rel": rels[c],
            "wrow": wrow,
            "iota": iota,
        })
    return in_maps, T, n_slots


TRACE = False          # set by test harness to capture an NTFF profile
TRACE_CORES = None
LAST_RESULTS = None    # BassKernelResults of the most recent run


def kernel(query, key, attn_kernel, targets):
    global LAST_RESULTS
    query = np.asarray(query, dtype=np.float32)
    key = np.asarray(key, dtype=np.float32)
    attn_kernel = np.asarray(attn_kernel, dtype=np.float32)
    targets = np.asarray(targets, dtype=np.int32)

    _ensure_imports()
    from concourse.bass_utils import run_bass_kernel_spmd

    in_maps, T, n_slots = _host_arrays(query, key, attn_kernel, targets)
    n_win = len(T)
    out_rows = n_win * WIN_NODES
    nc = build_nc(T, n_slots, out_rows)
    res = run_bass_kernel_spmd(
        nc, in_maps, list(range(N_CORES)),
        trace=TRACE, trace_cores=TRACE_CORES,
    )
    LAST_RESULTS = res
    shards = [res.results[c]["out"][:NODES_PER_CORE] for c in range(N_CORES)]
    out = np.concatenate(shards, axis=0).astype(np.float32)

    deg = np.bincount(targets, minlength=N_NODES)
    out[deg == 0] = 0.0
    return out


# revision 12
# speedup vs baseline: 1.1430x; 1.1430x over previous
"""GATv2 attention-pool kernel for 8 Trainium2 NeuronCores.

Algorithm
---------
Reference computes, per edge e with target node t(e):
    feats = q + k                                   [E, 64]
    logits[e,h] = sum_c feats[e,h*8+c] * A[c,h]     [E, 8]
    attn = segment_softmax(logits, targets)         [E, 8]
    out[n] = relu(segment_sum(q * attn))            [N, 64]

Because logits are O(10), exp() never overflows fp32, so the segment-max
shift is unnecessary and softmax folds into two segment-SUMS that share
one pass:
    denom[n,h]  = sum_{e->n} exp(logits[e,h])
    pooled[n,:] = sum_{e->n} q[e,:] * exp(logits[e,h])
    out[n]      = relu(pooled[n]) / denom[n]        (denom > 0 always)

Distribution: edges are partitioned by target node (host-side sort), 100000
nodes split into 8 contiguous shards of 12500 -> all segment reductions are
core-local, no collectives.  Each shard is cut into 196 windows of 64 nodes;
a window's edges are padded to T_w * 128 slots (T_w identical across cores so
one SPMD program serves all 8 cores).  Per 128-edge subtile the device builds
a one-hot selector S[e, n_rel] = (rel[e] == n_rel) and accumulates
    psum[64, 72] += S^T @ [q*ex | ex]
on the PE across the window's subtiles, then divides / relus once per node.

Host work is index metadata + data layout only (argsort of targets, gather
of q/k rows into the sorted slot order); all floating-point math runs on
device.
"""

import os
import sys

import numpy as np

N_NODES = 100000
N_EDGES = 1600000
H = 8
C = 8
HC = H * C
N_CORES = 8
NODES_PER_CORE = N_NODES // N_CORES
WIN_NODES = 64
SUB = 128


def _ensure_imports():
    try:
        import concourse.bass  # noqa: F401
    except ImportError:
        for p in ("/opt/trn_rl_repo", "/root/.axon_site/_ro/trn_rl_repo"):
            if os.path.isdir(p) and p not in sys.path:
                sys.path.insert(0, p)


def preprocess(targets, n_nodes, n_cores, win_nodes):
    """Sort edges by target; compute per-window slot layout shared by cores.

    Returns (perms [n_cores, n_slots] edge ids, rels [n_cores, n_slots] f32
    rel-node-or--1, T [n_win] subtiles per window, n_slots).
    """
    nodes_per_core = n_nodes // n_cores
    wins_per_core = (nodes_per_core + win_nodes - 1) // win_nodes
    order = np.argsort(targets, kind="stable")
    tsorted = targets[order]

    bounds = np.empty(n_cores * wins_per_core + 1, dtype=np.int64)
    i = 0
    for c in range(n_cores):
        for w in range(wins_per_core):
            bounds[i] = c * nodes_per_core + w * win_nodes
            i += 1
    bounds[-1] = n_nodes
    starts = np.searchsorted(tsorted, bounds[:-1], side="left")
    ends = np.concatenate([starts[1:], [len(tsorted)]])
    counts = (ends - starts).reshape(n_cores, wins_per_core)

    T = np.maximum(1, (counts.max(axis=0) + SUB - 1) // SUB).astype(np.int64)
    slots_per_win = T * SUB
    win_slot_base = np.concatenate([[0], np.cumsum(slots_per_win)])
    n_slots = int(win_slot_base[-1])

    perms = np.zeros((n_cores, n_slots), dtype=np.int64)
    rels = np.full((n_cores, n_slots), -1.0, dtype=np.float32)
    for c in range(n_cores):
        for w in range(wins_per_core):
            j = c * wins_per_core + w
            e0, e1 = starts[j], ends[j]
            sb = win_slot_base[w]
            cnt = e1 - e0
            perms[c, sb:sb + cnt] = order[e0:e1]
            rels[c, sb:sb + cnt] = (
                tsorted[e0:e1] - (c * nodes_per_core + w * win_nodes)
            ).astype(np.float32)
    return perms, rels, T, n_slots


def build_nc(T, n_slots, out_rows):
    """Build the single SPMD Bass program for one core's shard."""
    _ensure_imports()
    import concourse.bacc as bacc
    import concourse.mybir as mybir
    import concourse.tile as tile

    f32 = mybir.dt.float32
    Tmax = int(max(T))
    n_win = len(T)

    # process windows in pairs: one set of wide tiles per group amortizes
    # DVE per-op overhead and doubles DMA transfer sizes
    groups = []
    wb = 0
    w = 0
    while w < n_win:
        pair = [(w, int(T[w]), wb)]
        wb += int(T[w]) * SUB
        w += 1
        if w < n_win:
            pair.append((w, int(T[w]), wb))
            wb += int(T[w]) * SUB
            w += 1
        groups.append(pair)
    Tgmax = max(sum(t for _, t, _ in g) for g in groups)

    nc = bacc.Bacc("TRN2", num_devices=N_CORES)
    qk = nc.declare_dram_parameter("qk", [n_slots, 2 * HC], f32, False)
    rel = nc.declare_dram_parameter("rel", [n_slots], f32, False)
    wrow = nc.declare_dram_parameter("wrow", [128, Tgmax * HC], f32, False)
    iota = nc.declare_dram_parameter("iota", [128, WIN_NODES], f32, False)
    out = nc.declare_dram_parameter("out", [out_rows, HC], f32, isOutput=True)

    AX = mybir.AxisListType
    OP = mybir.AluOpType
    AF = mybir.ActivationFunctionType
    MW = 2 * HC  # qk row width

    with tile.TileContext(nc) as tc:
        with (
            tc.tile_pool(name="const", bufs=1) as cpool,
            tc.tile_pool(name="qk", bufs=4) as qkpool,
            tc.tile_pool(name="mid", bufs=3) as midpool,
            tc.tile_pool(name="mm", bufs=4) as mmpool,
            tc.tile_pool(name="fin", bufs=3) as finpool,
            tc.tile_pool(name="psum", bufs=6, space="PSUM") as ppool,
        ):
            w_t = cpool.tile([128, Tgmax * HC], f32)
            nc.sync.dma_start(out=w_t[:], in_=wrow[:])
            io_t = cpool.tile([128, WIN_NODES], f32)
            nc.sync.dma_start(out=io_t[:], in_=iota[:])

            for pair in groups:
                Tg = sum(t for _, t, _ in pair)
                fd = Tg * HC

                qk_t = qkpool.tile([128, Tg * MW], f32, tag="qk")
                r_t = qkpool.tile([128, Tg], f32, tag="r")
                off = 0
                for _, Tw, wbase in pair:
                    nsl = Tw * SUB
                    nc.sync.dma_start(
                        out=qk_t[:, off * MW:(off + Tw) * MW],
                        in_=qk[wbase:wbase + nsl, :].rearrange(
                            "(p t) c -> p (t c)", p=128),
                    )
                    nc.sync.dma_start(
                        out=r_t[:, off:off + Tw],
                        in_=rel[wbase:wbase + nsl].rearrange(
                            "(p t) -> p t", p=128),
                    )
                    off += Tw

                qk3 = qk_t[:].rearrange("p (t c) -> p t c", c=MW)

                # feats = q + k   (GpSimd)
                f_t = midpool.tile([128, fd], f32, tag="f")
                nc.gpsimd.tensor_add(
                    f_t[:], qk3[:, :, 0:HC], qk3[:, :, HC:MW]
                )

                # S one-hot: (rel == iota)
                s_t = mmpool.tile([128, Tg, WIN_NODES], f32, tag="S")
                nc.vector.tensor_tensor(
                    out=s_t[:],
                    in0=r_t[:, :, None].to_broadcast([128, Tg, WIN_NODES]),
                    in1=io_t[:, None, :].to_broadcast([128, Tg, WIN_NODES]),
                    op=OP.is_equal,
                )

                # wf = feats * Wrow ; logits = sum_c wf (split DVE/GpSimd)
                wf_t = midpool.tile([128, fd], f32, tag="wf")
                fd2 = (fd // 2) & ~63
                if fd2:
                    nc.gpsimd.tensor_mul(
                        wf_t[:, :fd2], f_t[:, :fd2], w_t[:, :fd2])
                nc.vector.tensor_mul(
                    wf_t[:, fd2:fd], f_t[:, fd2:fd], w_t[:, fd2:fd])
                lg_t = midpool.tile([128, Tg * H], f32, tag="lg")
                nc.vector.tensor_reduce(
                    out=lg_t[:],
                    in_=wf_t[:].rearrange("p (t h c) -> p (t h) c", h=H, c=C),
                    axis=AX.X,
                    op=OP.add,
                )

                # M = [q*ex | ex]
                m_t = mmpool.tile([128, Tg, H * C + H], f32, tag="M")
                nc.scalar.activation(
                    out=m_t[:, :, HC:HC + H],
                    in_=lg_t[:].rearrange("p (t h) -> p t h", h=H),
                    func=AF.Exp,
                )
                nc.vector.tensor_mul(
                    m_t[:, :, 0:HC].rearrange("p t (h c) -> p t h c", h=H),
                    qk3[:, :, 0:HC].rearrange("p t (h c) -> p t h c", h=H),
                    m_t[:, :, HC:HC + H, None].to_broadcast([128, Tg, H, C]),
                )

                # psum[64, 72*win] += S_g^T @ M_g  per window in the pair
                p_t = ppool.tile([WIN_NODES, len(pair) * (HC + H)], f32)
                off = 0
                for wi, (_, Tw, _) in enumerate(pair):
                    pcols = slice(wi * (HC + H), wi * (HC + H) + HC + H)
                    for g in range(Tw):
                        nc.tensor.matmul(
                            p_t[:, pcols],
                            lhsT=s_t[:, off + g, :],
                            rhs=m_t[:, off + g, :],
                            start=(g == 0),
                            stop=(g == Tw - 1),
                        )
                    off += Tw

                # out = relu(pooled) / denom, batched over the pair
                nw = len(pair)
                p3 = p_t[:].rearrange("p (w j) -> p w j", j=HC + H)
                relu_t = finpool.tile([WIN_NODES, nw, HC], f32, tag="relu")
                nc.scalar.activation(relu_t[:], p3[:, :, 0:HC], func=AF.Relu)
                rc_t = finpool.tile([WIN_NODES, nw, H], f32, tag="rc")
                nc.vector.reciprocal(rc_t[:], p3[:, :, HC:HC + H])
                o_t = finpool.tile([WIN_NODES, nw, HC], f32, tag="o")
                nc.vector.tensor_mul(
                    o_t[:].rearrange("p w (h c) -> p w h c", h=H),
                    relu_t[:].rearrange("p w (h c) -> p w h c", h=H),
                    rc_t[:, :, :, None].to_broadcast(
                        [WIN_NODES, nw, H, C]),
                )
                w0 = pair[0][0]
                nc.sync.dma_start(
                    out=out[w0 * WIN_NODES:(w0 + nw) * WIN_NODES, :]
                    .rearrange("(w p) c -> p w c", w=nw),
                    in_=o_t[:],
                )

    nc.finalize()
    return nc


def _host_arrays(query, key, attn_kernel, targets):
    perms, rels, T, n_slots = preprocess(
        targets, N_NODES, N_CORES, WIN_NODES
    )
    # widest 2-window group determines the resident W tile width
    Tg = [int(T[i]) + (int(T[i + 1]) if i + 1 < len(T) else 0)
          for i in range(0, len(T), 2)]
    Tgmax = max(Tg)
    wrow_1 = np.ascontiguousarray(attn_kernel.T).reshape(-1)  # [h*8+c] = A[c,h]
    wrow = np.tile(wrow_1, (128, Tgmax)).astype(np.float32)
    iota = np.tile(
        np.arange(WIN_NODES, dtype=np.float32), (128, 1)
    )
    in_maps = []
    for c in range(N_CORES):
        qkc = np.empty((n_slots, 2 * HC), dtype=np.float32)
        qkc[:, :HC] = query[perms[c]]
        qkc[:, HC:] = key[perms[c]]
        in_maps.append({
            "qk": qkc,
            "rel": rels[c],
            "wrow": wrow,
            "iota": iota,
        })
    return in_maps, T, n_slots


TRACE = False          # set by test harness to capture an NTFF profile
TRACE_CORES = None
LAST_RESULTS = None    # BassKernelResults of the most recent run


def kernel(query, key, attn_kernel, targets):
    global LAST_RESULTS
    query = np.asarray(query, dtype=np.float32)
    key = np.asarray(key, dtype=np.float32)
    attn_kernel = np.asarray(attn_kernel, dtype=np.float32)
    targets = np.asarray(targets, dtype=np.int32)

    _ensure_imports()
    from concourse.bass_utils import run_bass_kernel_spmd

    in_maps, T, n_slots = _host_arrays(query, key, attn_kernel, targets)
    n_win = len(T)
    out_rows = n_win * WIN_NODES
    nc = build_nc(T, n_slots, out_rows)
    res = run_bass_kernel_spmd(
        nc, in_maps, list(range(N_CORES)),
        trace=TRACE, trace_cores=TRACE_CORES,
    )
    LAST_RESULTS = res
    shards = [res.results[c]["out"][:NODES_PER_CORE] for c in range(N_CORES)]
    out = np.concatenate(shards, axis=0).astype(np.float32)

    deg = np.bincount(targets, minlength=N_NODES)
    out[deg == 0] = 0.0
    return out


# revision 13
# speedup vs baseline: 1.2510x; 1.0945x over previous
"""GATv2 attention-pool kernel for 8 Trainium2 NeuronCores.

Algorithm
---------
Reference computes, per edge e with target node t(e):
    feats = q + k                                   [E, 64]
    logits[e,h] = sum_c feats[e,h*8+c] * A[c,h]     [E, 8]
    attn = segment_softmax(logits, targets)         [E, 8]
    out[n] = relu(segment_sum(q * attn))            [N, 64]

Because logits are O(10), exp() never overflows fp32, so the segment-max
shift is unnecessary and softmax folds into two segment-SUMS that share
one pass:
    denom[n,h]  = sum_{e->n} exp(logits[e,h])
    pooled[n,:] = sum_{e->n} q[e,:] * exp(logits[e,h])
    out[n]      = relu(pooled[n]) / denom[n]        (denom > 0 always)

Distribution: edges are partitioned by target node (host-side sort), 100000
nodes split into 8 contiguous shards of 12500 -> all segment reductions are
core-local, no collectives.  Each shard is cut into 196 windows of 64 nodes;
a window's edges are padded to T_w * 128 slots (T_w identical across cores so
one SPMD program serves all 8 cores).  Per 128-edge subtile the device builds
a one-hot selector S[e, n_rel] = (rel[e] == n_rel) and accumulates
    psum[64, 72] += S^T @ [q*ex | ex]
on the PE across the window's subtiles, then divides / relus once per node.

Host work is index metadata + data layout only (argsort of targets, gather
of q/k rows into the sorted slot order); all floating-point math runs on
device.
"""

import os
import sys

import numpy as np

N_NODES = 100000
N_EDGES = 1600000
H = 8
C = 8
HC = H * C
N_CORES = 8
NODES_PER_CORE = N_NODES // N_CORES
WIN_NODES = 64
SUB = 128


def _ensure_imports():
    try:
        import concourse.bass  # noqa: F401
    except ImportError:
        for p in ("/opt/trn_rl_repo", "/root/.axon_site/_ro/trn_rl_repo"):
            if os.path.isdir(p) and p not in sys.path:
                sys.path.insert(0, p)


def preprocess(targets, n_nodes, n_cores, win_nodes):
    """Sort edges by target; compute per-window slot layout shared by cores.

    Returns (perms [n_cores, n_slots] edge ids, rels [n_cores, n_slots] f32
    rel-node-or--1, T [n_win] subtiles per window, n_slots).
    """
    nodes_per_core = n_nodes // n_cores
    wins_per_core = (nodes_per_core + win_nodes - 1) // win_nodes
    order = np.argsort(targets, kind="stable")
    tsorted = targets[order]

    bounds = np.empty(n_cores * wins_per_core + 1, dtype=np.int64)
    i = 0
    for c in range(n_cores):
        for w in range(wins_per_core):
            bounds[i] = c * nodes_per_core + w * win_nodes
            i += 1
    bounds[-1] = n_nodes
    starts = np.searchsorted(tsorted, bounds[:-1], side="left")
    ends = np.concatenate([starts[1:], [len(tsorted)]])
    counts = (ends - starts).reshape(n_cores, wins_per_core)

    T = np.maximum(1, (counts.max(axis=0) + SUB - 1) // SUB).astype(np.int64)
    slots_per_win = T * SUB
    win_slot_base = np.concatenate([[0], np.cumsum(slots_per_win)])
    n_slots = int(win_slot_base[-1])

    perms = np.zeros((n_cores, n_slots), dtype=np.int64)
    rels = np.full((n_cores, n_slots), -1.0, dtype=np.float32)
    for c in range(n_cores):
        for w in range(wins_per_core):
            j = c * wins_per_core + w
            e0, e1 = starts[j], ends[j]
            sb = win_slot_base[w]
            cnt = e1 - e0
            perms[c, sb:sb + cnt] = order[e0:e1]
            rels[c, sb:sb + cnt] = (
                tsorted[e0:e1] - (c * nodes_per_core + w * win_nodes)
            ).astype(np.float32)
    return perms, rels, T, n_slots


def build_nc(T, n_slots, out_rows):
    """Build the single SPMD Bass program for one core's shard."""
    _ensure_imports()
    import concourse.bacc as bacc
    import concourse.mybir as mybir
    import concourse.tile as tile

    f32 = mybir.dt.float32
    Tmax = int(max(T))
    n_win = len(T)

    # process windows in pairs: one set of wide tiles per group amortizes
    # DVE per-op overhead and doubles DMA transfer sizes
    groups = []
    wb = 0
    w = 0
    while w < n_win:
        pair = [(w, int(T[w]), wb)]
        wb += int(T[w]) * SUB
        w += 1
        if w < n_win:
            pair.append((w, int(T[w]), wb))
            wb += int(T[w]) * SUB
            w += 1
        groups.append(pair)
    Tgmax = max(sum(t for _, t, _ in g) for g in groups)

    nc = bacc.Bacc("TRN2", num_devices=N_CORES)
    qk = nc.declare_dram_parameter("qk", [n_slots, 2 * HC], f32, False)
    rel = nc.declare_dram_parameter("rel", [n_slots], f32, False)
    wrow = nc.declare_dram_parameter("wrow", [128, Tgmax * HC], f32, False)
    iota = nc.declare_dram_parameter("iota", [128, WIN_NODES], f32, False)
    out = nc.declare_dram_parameter("out", [out_rows, HC], f32, isOutput=True)

    AX = mybir.AxisListType
    OP = mybir.AluOpType
    AF = mybir.ActivationFunctionType
    MW = 2 * HC  # qk row width

    with tile.TileContext(nc) as tc:
        with (
            tc.tile_pool(name="const", bufs=1) as cpool,
            tc.tile_pool(name="qk", bufs=4) as qkpool,
            tc.tile_pool(name="mid", bufs=3) as midpool,
            tc.tile_pool(name="mm", bufs=4) as mmpool,
            tc.tile_pool(name="fin", bufs=3) as finpool,
            tc.tile_pool(name="psum", bufs=6, space="PSUM") as ppool,
        ):
            w_t = cpool.tile([128, Tgmax * HC], f32)
            nc.sync.dma_start(out=w_t[:], in_=wrow[:])
            io_t = cpool.tile([128, WIN_NODES], f32)
            nc.sync.dma_start(out=io_t[:], in_=iota[:])

            for pair in groups:
                Tg = sum(t for _, t, _ in pair)
                fd = Tg * HC

                qk_t = qkpool.tile([128, Tg * MW], f32, tag="qk")
                r_t = qkpool.tile([128, Tg], f32, tag="r")
                off = 0
                for _, Tw, wbase in pair:
                    nsl = Tw * SUB
                    nc.sync.dma_start(
                        out=qk_t[:, off * MW:(off + Tw) * MW],
                        in_=qk[wbase:wbase + nsl, :].rearrange(
                            "(p t) c -> p (t c)", p=128),
                    )
                    nc.sync.dma_start(
                        out=r_t[:, off:off + Tw],
                        in_=rel[wbase:wbase + nsl].rearrange(
                            "(p t) -> p t", p=128),
                    )
                    off += Tw

                qk3 = qk_t[:].rearrange("p (t c) -> p t c", c=MW)

                # feats = q + k   (GpSimd)
                f_t = midpool.tile([128, fd], f32, tag="f")
                nc.gpsimd.tensor_add(
                    f_t[:], qk3[:, :, 0:HC], qk3[:, :, HC:MW]
                )

                # S one-hot: (rel == iota)
                s_t = mmpool.tile([128, Tg, WIN_NODES], f32, tag="S")
                nc.vector.tensor_tensor(
                    out=s_t[:],
                    in0=r_t[:, :, None].to_broadcast([128, Tg, WIN_NODES]),
                    in1=io_t[:, None, :].to_broadcast([128, Tg, WIN_NODES]),
                    op=OP.is_equal,
                )

                # wf = feats * Wrow ; logits = sum_c wf  (DVE)
                wf_t = midpool.tile([128, fd], f32, tag="wf")
                nc.vector.tensor_mul(wf_t[:], f_t[:], w_t[:, :fd])
                lg_t = midpool.tile([128, Tg * H], f32, tag="lg")
                nc.vector.tensor_reduce(
                    out=lg_t[:],
                    in_=wf_t[:].rearrange("p (t h c) -> p (t h) c", h=H, c=C),
                    axis=AX.X,
                    op=OP.add,
                )

                # M = [q*ex | ex]
                m_t = mmpool.tile([128, Tg, H * C + H], f32, tag="M")
                nc.scalar.activation(
                    out=m_t[:, :, HC:HC + H],
                    in_=lg_t[:].rearrange("p (t h) -> p t h", h=H),
                    func=AF.Exp,
                )
                nc.vector.tensor_mul(
                    m_t[:, :, 0:HC].rearrange("p t (h c) -> p t h c", h=H),
                    qk3[:, :, 0:HC].rearrange("p t (h c) -> p t h c", h=H),
                    m_t[:, :, HC:HC + H, None].to_broadcast([128, Tg, H, C]),
                )

                # psum[64, 72*win] += S_g^T @ M_g  per window in the pair
                p_t = ppool.tile([WIN_NODES, len(pair) * (HC + H)], f32)
                off = 0
                for wi, (_, Tw, _) in enumerate(pair):
                    pcols = slice(wi * (HC + H), wi * (HC + H) + HC + H)
                    for g in range(Tw):
                        nc.tensor.matmul(
                            p_t[:, pcols],
                            lhsT=s_t[:, off + g, :],
                            rhs=m_t[:, off + g, :],
                            start=(g == 0),
                            stop=(g == Tw - 1),
                        )
                    off += Tw

                # out = relu(pooled) / denom, batched over the pair
                nw = len(pair)
                p3 = p_t[:].rearrange("p (w j) -> p w j", j=HC + H)
                relu_t = finpool.tile([WIN_NODES, nw, HC], f32, tag="relu")
                nc.scalar.activation(relu_t[:], p3[:, :, 0:HC], func=AF.Relu)
                rc_t = finpool.tile([WIN_NODES, nw, H], f32, tag="rc")
                nc.vector.reciprocal(rc_t[:], p3[:, :, HC:HC + H])
                o_t = finpool.tile([WIN_NODES, nw, HC], f32, tag="o")
                nc.vector.tensor_mul(
                    o_t[:].rearrange("p w (h c) -> p w h c", h=H),
                    relu_t[:].rearrange("p w (h c) -> p w h c", h=H),
                    rc_t[:, :, :, None].to_broadcast(
                        [WIN_NODES, nw, H, C]),
                )
                w0 = pair[0][0]
                nc.sync.dma_start(
                    out=out[w0 * WIN_NODES:(w0 + nw) * WIN_NODES, :]
                    .rearrange("(w p) c -> p w c", w=nw),
                    in_=o_t[:],
                )

    nc.finalize()
    return nc


def _host_arrays(query, key, attn_kernel, targets):
    perms, rels, T, n_slots = preprocess(
        targets, N_NODES, N_CORES, WIN_NODES
    )
    # widest 2-window group determines the resident W tile width
    Tg = [int(T[i]) + (int(T[i + 1]) if i + 1 < len(T) else 0)
          for i in range(0, len(T), 2)]
    Tgmax = max(Tg)
    wrow_1 = np.ascontiguousarray(attn_kernel.T).reshape(-1)  # [h*8+c] = A[c,h]
    wrow = np.tile(wrow_1, (128, Tgmax)).astype(np.float32)
    iota = np.tile(
        np.arange(WIN_NODES, dtype=np.float32), (128, 1)
    )
    in_maps = []
    for c in range(N_CORES):
        qkc = np.empty((n_slots, 2 * HC), dtype=np.float32)
        qkc[:, :HC] = query[perms[c]]
        qkc[:, HC:] = key[perms[c]]
        in_maps.append({
            "qk": qkc,
            "rel": rels[c],
            "wrow": wrow,
            "iota": iota,
        })
    return in_maps, T, n_slots


TRACE = False          # set by test harness to capture an NTFF profile
TRACE_CORES = None
LAST_RESULTS = None    # BassKernelResults of the most recent run


def kernel(query, key, attn_kernel, targets):
    global LAST_RESULTS
    query = np.asarray(query, dtype=np.float32)
    key = np.asarray(key, dtype=np.float32)
    attn_kernel = np.asarray(attn_kernel, dtype=np.float32)
    targets = np.asarray(targets, dtype=np.int32)

    _ensure_imports()
    from concourse.bass_utils import run_bass_kernel_spmd

    in_maps, T, n_slots = _host_arrays(query, key, attn_kernel, targets)
    n_win = len(T)
    out_rows = n_win * WIN_NODES
    nc = build_nc(T, n_slots, out_rows)
    res = run_bass_kernel_spmd(
        nc, in_maps, list(range(N_CORES)),
        trace=TRACE, trace_cores=TRACE_CORES,
    )
    LAST_RESULTS = res
    shards = [res.results[c]["out"][:NODES_PER_CORE] for c in range(N_CORES)]
    out = np.concatenate(shards, axis=0).astype(np.float32)

    deg = np.bincount(targets, minlength=N_NODES)
    out[deg == 0] = 0.0
    return out


# revision 17
# speedup vs baseline: 1.2803x; 1.0234x over previous
"""GATv2 attention-pool kernel for 8 Trainium2 NeuronCores.

Algorithm
---------
Reference computes, per edge e with target node t(e):
    feats = q + k                                   [E, 64]
    logits[e,h] = sum_c feats[e,h*8+c] * A[c,h]     [E, 8]
    attn = segment_softmax(logits, targets)         [E, 8]
    out[n] = relu(segment_sum(q * attn))            [N, 64]

Because logits are O(10), exp() never overflows fp32, so the segment-max
shift is unnecessary and softmax folds into two segment-SUMS that share
one pass:
    denom[n,h]  = sum_{e->n} exp(logits[e,h])
    pooled[n,:] = sum_{e->n} q[e,:] * exp(logits[e,h])
    out[n]      = relu(pooled[n]) / denom[n]        (denom > 0 always)

Distribution: edges are partitioned by target node (host-side sort), 100000
nodes split into 8 contiguous shards of 12500 -> all segment reductions are
core-local, no collectives.  Each shard is cut into 196 windows of 64 nodes;
a window's edges are padded to T_w * 128 slots (T_w identical across cores so
one SPMD program serves all 8 cores).  Per 128-edge subtile the device builds
a one-hot selector S[e, n_rel] = (rel[e] == n_rel) and accumulates
    psum[64, 72] += S^T @ [q*ex | ex]
on the PE across the window's subtiles, then divides / relus once per node.

Host work is index metadata + data layout only (argsort of targets, gather
of q/k rows into the sorted slot order); all floating-point math runs on
device.
"""

import os
import sys

import numpy as np

N_NODES = 100000
N_EDGES = 1600000
H = 8
C = 8
HC = H * C
N_CORES = 8
NODES_PER_CORE = N_NODES // N_CORES
WIN_NODES = 64
SUB = 128


def _ensure_imports():
    try:
        import concourse.bass  # noqa: F401
    except ImportError:
        for p in ("/opt/trn_rl_repo", "/root/.axon_site/_ro/trn_rl_repo"):
            if os.path.isdir(p) and p not in sys.path:
                sys.path.insert(0, p)


def preprocess(targets, n_nodes, n_cores, win_nodes):
    """Sort edges by target; compute per-window slot layout shared by cores.

    Returns (perms [n_cores, n_slots] edge ids, rels [n_cores, n_slots] f32
    rel-node-or--1, T [n_win] subtiles per window, n_slots).
    """
    nodes_per_core = n_nodes // n_cores
    wins_per_core = (nodes_per_core + win_nodes - 1) // win_nodes
    order = np.argsort(targets, kind="stable")
    tsorted = targets[order]

    bounds = np.empty(n_cores * wins_per_core + 1, dtype=np.int64)
    i = 0
    for c in range(n_cores):
        for w in range(wins_per_core):
            bounds[i] = c * nodes_per_core + w * win_nodes
            i += 1
    bounds[-1] = n_nodes
    starts = np.searchsorted(tsorted, bounds[:-1], side="left")
    ends = np.concatenate([starts[1:], [len(tsorted)]])
    counts = (ends - starts).reshape(n_cores, wins_per_core)

    T = np.maximum(1, (counts.max(axis=0) + SUB - 1) // SUB).astype(np.int64)
    slots_per_win = T * SUB
    win_slot_base = np.concatenate([[0], np.cumsum(slots_per_win)])
    n_slots = int(win_slot_base[-1])

    perms = np.zeros((n_cores, n_slots), dtype=np.int64)
    rels = np.full((n_cores, n_slots), -1.0, dtype=np.float32)
    for c in range(n_cores):
        for w in range(wins_per_core):
            j = c * wins_per_core + w
            e0, e1 = starts[j], ends[j]
            sb = win_slot_base[w]
            cnt = e1 - e0
            perms[c, sb:sb + cnt] = order[e0:e1]
            rels[c, sb:sb + cnt] = (
                tsorted[e0:e1] - (c * nodes_per_core + w * win_nodes)
            ).astype(np.float32)
    return perms, rels, T, n_slots


def build_nc(T, n_slots, out_rows):
    """Build the single SPMD Bass program for one core's shard."""
    _ensure_imports()
    import concourse.bacc as bacc
    import concourse.mybir as mybir
    import concourse.tile as tile

    f32 = mybir.dt.float32
    Tmax = int(max(T))
    n_win = len(T)

    # process windows in pairs: one set of wide tiles per group amortizes
    # DVE per-op overhead and doubles DMA transfer sizes
    groups = []
    wb = 0
    w = 0
    while w < n_win:
        pair = [(w, int(T[w]), wb)]
        wb += int(T[w]) * SUB
        w += 1
        if w < n_win:
            pair.append((w, int(T[w]), wb))
            wb += int(T[w]) * SUB
            w += 1
        groups.append(pair)
    Tgmax = max(sum(t for _, t, _ in g) for g in groups)

    i16 = mybir.dt.int16
    bf16 = mybir.dt.bfloat16
    nc = bacc.Bacc("TRN2", num_devices=N_CORES)
    qk = nc.declare_dram_parameter("qk", [n_slots, 2 * HC], f32, False)
    rel = nc.declare_dram_parameter("rel", [n_slots], f32, False)
    wrow = nc.declare_dram_parameter("wrow", [128, Tgmax * HC], f32, False)
    iota16 = nc.declare_dram_parameter(
        "iota16", [128, Tgmax * WIN_NODES], i16, False)
    out = nc.declare_dram_parameter("out", [out_rows, HC], f32, isOutput=True)

    AX = mybir.AxisListType
    OP = mybir.AluOpType
    AF = mybir.ActivationFunctionType
    MW = 2 * HC  # qk row width

    with tile.TileContext(nc) as tc:
        with (
            tc.tile_pool(name="const", bufs=1) as cpool,
            tc.tile_pool(name="qk", bufs=4) as qkpool,
            tc.tile_pool(name="mid", bufs=3) as midpool,
            tc.tile_pool(name="mm", bufs=4) as mmpool,
            tc.tile_pool(name="fin", bufs=3) as finpool,
            tc.tile_pool(name="psum", bufs=6, space="PSUM") as ppool,
        ):
            w_t = cpool.tile([128, Tgmax * HC], f32)
            nc.sync.dma_start(out=w_t[:], in_=wrow[:])
            io_t = cpool.tile([128, Tgmax * WIN_NODES], i16)
            nc.sync.dma_start(out=io_t[:], in_=iota16[:])

            for pair in groups:
                Tg = sum(t for _, t, _ in pair)
                fd = Tg * HC

                qk_t = qkpool.tile([128, Tg * MW], f32, tag="qk")
                r_t = qkpool.tile([128, Tg], f32, tag="r")
                off = 0
                for _, Tw, wbase in pair:
                    nsl = Tw * SUB
                    nc.sync.dma_start(
                        out=qk_t[:, off * MW:(off + Tw) * MW],
                        in_=qk[wbase:wbase + nsl, :].rearrange(
                            "(p t) c -> p (t c)", p=128),
                    )
                    nc.sync.dma_start(
                        out=r_t[:, off:off + Tw],
                        in_=rel[wbase:wbase + nsl].rearrange(
                            "(p t) -> p t", p=128),
                    )
                    off += Tw

                qk3 = qk_t[:].rearrange("p (t c) -> p t c", c=MW)

                # feats = q + k   (GpSimd)
                f_t = midpool.tile([128, fd], f32, tag="f")
                nc.gpsimd.tensor_add(
                    f_t[:], qk3[:, :, 0:HC], qk3[:, :, HC:MW]
                )

                # S one-hot: (rel == iota).  ACT (own SBUF ports) expands rel
                # to int16 and upconverts the bf16 compare result to f32, so
                # DVE only pays a 2x-mode 16-bit IS_EQ.
                rr_t = mmpool.tile([128, Tg, WIN_NODES], i16, tag="rr")
                nc.scalar.activation(
                    out=rr_t[:],
                    in_=r_t[:, :, None].to_broadcast([128, Tg, WIN_NODES]),
                    func=AF.Copy,
                )
                sb_t = mmpool.tile([128, Tg, WIN_NODES], bf16, tag="Sb")
                nc.vector.tensor_tensor(
                    out=sb_t[:],
                    in0=rr_t[:],
                    in1=io_t[:, :Tg * WIN_NODES].rearrange(
                        "p (t n) -> p t n", n=WIN_NODES),
                    op=OP.is_equal,
                )
                s_t = mmpool.tile([128, Tg, WIN_NODES], f32, tag="S")
                nc.scalar.activation(out=s_t[:], in_=sb_t[:], func=AF.Copy)

                # wf = feats * Wrow ; logits = sum_c wf  (DVE)
                wf_t = midpool.tile([128, fd], f32, tag="wf")
                nc.vector.tensor_mul(wf_t[:], f_t[:], w_t[:, :fd])
                lg_t = midpool.tile([128, Tg * H], f32, tag="lg")
                nc.vector.tensor_reduce(
                    out=lg_t[:],
                    in_=wf_t[:].rearrange("p (t h c) -> p (t h) c", h=H, c=C),
                    axis=AX.X,
                    op=OP.add,
                )

                # M = [q*ex | ex]
                m_t = mmpool.tile([128, Tg, H * C + H], f32, tag="M")
                nc.scalar.activation(
                    out=m_t[:, :, HC:HC + H],
                    in_=lg_t[:].rearrange("p (t h) -> p t h", h=H),
                    func=AF.Exp,
                )
                nc.vector.tensor_mul(
                    m_t[:, :, 0:HC].rearrange("p t (h c) -> p t h c", h=H),
                    qk3[:, :, 0:HC].rearrange("p t (h c) -> p t h c", h=H),
                    m_t[:, :, HC:HC + H, None].to_broadcast([128, Tg, H, C]),
                )

                # psum[64, 72*win] += S_g^T @ M_g  per window in the pair
                p_t = ppool.tile([WIN_NODES, len(pair) * (HC + H)], f32)
                off = 0
                for wi, (_, Tw, _) in enumerate(pair):
                    pcols = slice(wi * (HC + H), wi * (HC + H) + HC + H)
                    for g in range(Tw):
                        nc.tensor.matmul(
                            p_t[:, pcols],
                            lhsT=s_t[:, off + g, :],
                            rhs=m_t[:, off + g, :],
                            start=(g == 0),
                            stop=(g == Tw - 1),
                        )
                    off += Tw

                # out = relu(pooled) / denom, batched over the pair
                nw = len(pair)
                p3 = p_t[:].rearrange("p (w j) -> p w j", j=HC + H)
                relu_t = finpool.tile([WIN_NODES, nw, HC], f32, tag="relu")
                nc.scalar.activation(relu_t[:], p3[:, :, 0:HC], func=AF.Relu)
                rc_t = finpool.tile([WIN_NODES, nw, H], f32, tag="rc")
                nc.vector.reciprocal(rc_t[:], p3[:, :, HC:HC + H])
                o_t = finpool.tile([WIN_NODES, nw, HC], f32, tag="o")
                nc.vector.tensor_mul(
                    o_t[:].rearrange("p w (h c) -> p w h c", h=H),
                    relu_t[:].rearrange("p w (h c) -> p w h c", h=H),
                    rc_t[:, :, :, None].to_broadcast(
                        [WIN_NODES, nw, H, C]),
                )
                w0 = pair[0][0]
                nc.sync.dma_start(
                    out=out[w0 * WIN_NODES:(w0 + nw) * WIN_NODES, :]
                    .rearrange("(w p) c -> p w c", w=nw),
                    in_=o_t[:],
                )

    nc.finalize()
    return nc


def _host_arrays(query, key, attn_kernel, targets):
    perms, rels, T, n_slots = preprocess(
        targets, N_NODES, N_CORES, WIN_NODES
    )
    # widest 2-window group determines the resident W tile width
    Tg = [int(T[i]) + (int(T[i + 1]) if i + 1 < len(T) else 0)
          for i in range(0, len(T), 2)]
    Tgmax = max(Tg)
    wrow_1 = np.ascontiguousarray(attn_kernel.T).reshape(-1)  # [h*8+c] = A[c,h]
    wrow = np.tile(wrow_1, (128, Tgmax)).astype(np.float32)
    iota16 = np.tile(
        np.arange(WIN_NODES, dtype=np.int16), (128, Tgmax)
    )
    in_maps = []
    for c in range(N_CORES):
        qkc = np.empty((n_slots, 2 * HC), dtype=np.float32)
        qkc[:, :HC] = query[perms[c]]
        qkc[:, HC:] = key[perms[c]]
        in_maps.append({
            "qk": qkc,
            "rel": rels[c],
            "wrow": wrow,
            "iota16": iota16,
        })
    return in_maps, T, n_slots


TRACE = False          # set by test harness to capture an NTFF profile
TRACE_CORES = None
LAST_RESULTS = None    # BassKernelResults of the most recent run


def kernel(query, key, attn_kernel, targets):
    global LAST_RESULTS
    query = np.asarray(query, dtype=np.float32)
    key = np.asarray(key, dtype=np.float32)
    attn_kernel = np.asarray(attn_kernel, dtype=np.float32)
    targets = np.asarray(targets, dtype=np.int32)

    _ensure_imports()
    from concourse.bass_utils import run_bass_kernel_spmd

    in_maps, T, n_slots = _host_arrays(query, key, attn_kernel, targets)
    n_win = len(T)
    out_rows = n_win * WIN_NODES
    nc = build_nc(T, n_slots, out_rows)
    res = run_bass_kernel_spmd(
        nc, in_maps, list(range(N_CORES)),
        trace=TRACE, trace_cores=TRACE_CORES,
    )
    LAST_RESULTS = res
    shards = [res.results[c]["out"][:NODES_PER_CORE] for c in range(N_CORES)]
    out = np.concatenate(shards, axis=0).astype(np.float32)

    deg = np.bincount(targets, minlength=N_NODES)
    out[deg == 0] = 0.0
    return out


# revision 18
# speedup vs baseline: 1.3161x; 1.0280x over previous
"""GATv2 attention-pool kernel for 8 Trainium2 NeuronCores.

Algorithm
---------
Reference computes, per edge e with target node t(e):
    feats = q + k                                   [E, 64]
    logits[e,h] = sum_c feats[e,h*8+c] * A[c,h]     [E, 8]
    attn = segment_softmax(logits, targets)         [E, 8]
    out[n] = relu(segment_sum(q * attn))            [N, 64]

Because logits are O(10), exp() never overflows fp32, so the segment-max
shift is unnecessary and softmax folds into two segment-SUMS that share
one pass:
    denom[n,h]  = sum_{e->n} exp(logits[e,h])
    pooled[n,:] = sum_{e->n} q[e,:] * exp(logits[e,h])
    out[n]      = relu(pooled[n]) / denom[n]        (denom > 0 always)

Distribution: edges are partitioned by target node (host-side sort), 100000
nodes split into 8 contiguous shards of 12500 -> all segment reductions are
core-local, no collectives.  Each shard is cut into 196 windows of 64 nodes;
a window's edges are padded to T_w * 128 slots (T_w identical across cores so
one SPMD program serves all 8 cores).  Per 128-edge subtile the device builds
a one-hot selector S[e, n_rel] = (rel[e] == n_rel) and accumulates
    psum[64, 72] += S^T @ [q*ex | ex]
on the PE across the window's subtiles, then divides / relus once per node.

Host work is index metadata + data layout only (argsort of targets, gather
of q/k rows into the sorted slot order); all floating-point math runs on
device.
"""

import os
import sys

import numpy as np

N_NODES = 100000
N_EDGES = 1600000
H = 8
C = 8
HC = H * C
N_CORES = 8
NODES_PER_CORE = N_NODES // N_CORES
WIN_NODES = 64
SUB = 128


def _ensure_imports():
    try:
        import concourse.bass  # noqa: F401
    except ImportError:
        for p in ("/opt/trn_rl_repo", "/root/.axon_site/_ro/trn_rl_repo"):
            if os.path.isdir(p) and p not in sys.path:
                sys.path.insert(0, p)


def preprocess(targets, n_nodes, n_cores, win_nodes):
    """Sort edges by target; compute per-window slot layout shared by cores.

    Returns (perms [n_cores, n_slots] edge ids, rels [n_cores, n_slots] f32
    rel-node-or--1, T [n_win] subtiles per window, n_slots).
    """
    nodes_per_core = n_nodes // n_cores
    wins_per_core = (nodes_per_core + win_nodes - 1) // win_nodes
    order = np.argsort(targets, kind="stable")
    tsorted = targets[order]

    bounds = np.empty(n_cores * wins_per_core + 1, dtype=np.int64)
    i = 0
    for c in range(n_cores):
        for w in range(wins_per_core):
            bounds[i] = c * nodes_per_core + w * win_nodes
            i += 1
    bounds[-1] = n_nodes
    starts = np.searchsorted(tsorted, bounds[:-1], side="left")
    ends = np.concatenate([starts[1:], [len(tsorted)]])
    counts = (ends - starts).reshape(n_cores, wins_per_core)

    T = np.maximum(1, (counts.max(axis=0) + SUB - 1) // SUB).astype(np.int64)
    slots_per_win = T * SUB
    win_slot_base = np.concatenate([[0], np.cumsum(slots_per_win)])
    n_slots = int(win_slot_base[-1])

    perms = np.zeros((n_cores, n_slots), dtype=np.int64)
    rels = np.full((n_cores, n_slots), -1.0, dtype=np.float32)
    for c in range(n_cores):
        for w in range(wins_per_core):
            j = c * wins_per_core + w
            e0, e1 = starts[j], ends[j]
            sb = win_slot_base[w]
            cnt = e1 - e0
            perms[c, sb:sb + cnt] = order[e0:e1]
            rels[c, sb:sb + cnt] = (
                tsorted[e0:e1] - (c * nodes_per_core + w * win_nodes)
            ).astype(np.float32)
    return perms, rels, T, n_slots


def build_nc(T, n_slots, out_rows):
    """Build the single SPMD Bass program for one core's shard."""
    _ensure_imports()
    import concourse.bacc as bacc
    import concourse.mybir as mybir
    import concourse.tile as tile

    f32 = mybir.dt.float32
    Tmax = int(max(T))
    n_win = len(T)

    # process windows in pairs: one set of wide tiles per group amortizes
    # DVE per-op overhead and doubles DMA transfer sizes
    groups = []
    wb = 0
    w = 0
    while w < n_win:
        pair = [(w, int(T[w]), wb)]
        wb += int(T[w]) * SUB
        w += 1
        if w < n_win:
            pair.append((w, int(T[w]), wb))
            wb += int(T[w]) * SUB
            w += 1
        groups.append(pair)
    Tgmax = max(sum(t for _, t, _ in g) for g in groups)

    i16 = mybir.dt.int16
    bf16 = mybir.dt.bfloat16
    nc = bacc.Bacc("TRN2", num_devices=N_CORES)
    qk = nc.declare_dram_parameter("qk", [n_slots, 2 * HC], f32, False)
    rel = nc.declare_dram_parameter("rel", [n_slots], f32, False)
    wrow = nc.declare_dram_parameter("wrow", [128, Tgmax * HC], f32, False)
    iota16 = nc.declare_dram_parameter(
        "iota16", [128, Tgmax * WIN_NODES], i16, False)
    out = nc.declare_dram_parameter("out", [out_rows, HC], f32, isOutput=True)

    AX = mybir.AxisListType
    OP = mybir.AluOpType
    AF = mybir.ActivationFunctionType
    MW = 2 * HC  # qk row width

    with tile.TileContext(nc) as tc:
        with (
            tc.tile_pool(name="const", bufs=1) as cpool,
            tc.tile_pool(name="qk", bufs=4) as qkpool,
            tc.tile_pool(name="mid", bufs=3) as midpool,
            tc.tile_pool(name="mm", bufs=4) as mmpool,
            tc.tile_pool(name="fin", bufs=3) as finpool,
            tc.tile_pool(name="psum", bufs=6, space="PSUM") as ppool,
        ):
            w_t = cpool.tile([128, Tgmax * HC], f32)
            nc.sync.dma_start(out=w_t[:], in_=wrow[:])
            io_t = cpool.tile([128, Tgmax * WIN_NODES], i16)
            nc.sync.dma_start(out=io_t[:], in_=iota16[:])

            # software-pipelined by one group: the S-path and logits of
            # group i+1 are emitted between group i's exp/wq and its
            # epilogue, so ACT's FIFO runs exp_i, rr_{i+1}, sup_{i+1},
            # relu_i and never makes DVE wait on a long COPY.
            st = {}

            def emit_load(pair):
                Tg = sum(t for _, t, _ in pair)
                fd = Tg * HC
                qk_t = qkpool.tile([128, Tg * MW], f32, tag="qk")
                r_t = qkpool.tile([128, Tg], f32, tag="r")
                off = 0
                for _, Tw, wbase in pair:
                    nsl = Tw * SUB
                    nc.sync.dma_start(
                        out=qk_t[:, off * MW:(off + Tw) * MW],
                        in_=qk[wbase:wbase + nsl, :].rearrange(
                            "(p t) c -> p (t c)", p=128),
                    )
                    nc.sync.dma_start(
                        out=r_t[:, off:off + Tw],
                        in_=rel[wbase:wbase + nsl].rearrange(
                            "(p t) -> p t", p=128),
                    )
                    off += Tw
                qk3 = qk_t[:].rearrange("p (t c) -> p t c", c=MW)
                f_t = midpool.tile([128, fd], f32, tag="f")
                nc.gpsimd.tensor_add(
                    f_t[:], qk3[:, :, 0:HC], qk3[:, :, HC:MW])
                return {"pair": pair, "Tg": Tg, "fd": fd, "qk3": qk3,
                        "f": f_t, "r": r_t}

            def emit_spath(s):
                Tg = s["Tg"]
                rr_t = mmpool.tile([128, Tg, WIN_NODES], i16, tag="rr")
                nc.scalar.activation(
                    out=rr_t[:],
                    in_=s["r"][:, :, None].to_broadcast(
                        [128, Tg, WIN_NODES]),
                    func=AF.Copy,
                )
                sb_t = mmpool.tile([128, Tg, WIN_NODES], bf16, tag="Sb")
                nc.vector.tensor_tensor(
                    out=sb_t[:],
                    in0=rr_t[:],
                    in1=io_t[:, :Tg * WIN_NODES].rearrange(
                        "p (t n) -> p t n", n=WIN_NODES),
                    op=OP.is_equal,
                )
                s_t = mmpool.tile([128, Tg, WIN_NODES], f32, tag="S")
                nc.scalar.activation(out=s_t[:], in_=sb_t[:], func=AF.Copy)
                s["S"] = s_t

            def emit_logits(s):
                Tg, fd = s["Tg"], s["fd"]
                wf_t = midpool.tile([128, fd], f32, tag="wf")
                nc.vector.tensor_mul(wf_t[:], s["f"][:], w_t[:, :fd])
                lg_t = midpool.tile([128, Tg * H], f32, tag="lg")
                nc.vector.tensor_reduce(
                    out=lg_t[:],
                    in_=wf_t[:].rearrange(
                        "p (t h c) -> p (t h) c", h=H, c=C),
                    axis=AX.X,
                    op=OP.add,
                )
                s["lg"] = lg_t

            def emit_exp_wq_mm(s):
                Tg = s["Tg"]
                m_t = mmpool.tile([128, Tg, H * C + H], f32, tag="M")
                nc.scalar.activation(
                    out=m_t[:, :, HC:HC + H],
                    in_=s["lg"][:].rearrange("p (t h) -> p t h", h=H),
                    func=AF.Exp,
                )
                nc.vector.tensor_mul(
                    m_t[:, :, 0:HC].rearrange("p t (h c) -> p t h c", h=H),
                    s["qk3"][:, :, 0:HC].rearrange(
                        "p t (h c) -> p t h c", h=H),
                    m_t[:, :, HC:HC + H, None].to_broadcast(
                        [128, Tg, H, C]),
                )
                pair = s["pair"]
                p_t = ppool.tile([WIN_NODES, len(pair) * (HC + H)], f32)
                off = 0
                for wi, (_, Tw, _) in enumerate(pair):
                    pcols = slice(wi * (HC + H), wi * (HC + H) + HC + H)
                    for g in range(Tw):
                        nc.tensor.matmul(
                            p_t[:, pcols],
                            lhsT=s["S"][:, off + g, :],
                            rhs=m_t[:, off + g, :],
                            start=(g == 0),
                            stop=(g == Tw - 1),
                        )
                    off += Tw
                s["psum"] = p_t

            def emit_epilogue(s):
                pair = s["pair"]
                nw = len(pair)
                p3 = s["psum"][:].rearrange("p (w j) -> p w j", j=HC + H)
                rc_t = finpool.tile([WIN_NODES, nw, H], f32, tag="rc")
                nc.vector.reciprocal(rc_t[:], p3[:, :, HC:HC + H])
                relu_t = finpool.tile([WIN_NODES, nw, HC], f32, tag="relu")
                nc.scalar.activation(
                    relu_t[:], p3[:, :, 0:HC], func=AF.Relu)
                o_t = finpool.tile([WIN_NODES, nw, HC], f32, tag="o")
                nc.vector.tensor_mul(
                    o_t[:].rearrange("p w (h c) -> p w h c", h=H),
                    relu_t[:].rearrange("p w (h c) -> p w h c", h=H),
                    rc_t[:, :, :, None].to_broadcast(
                        [WIN_NODES, nw, H, C]),
                )
                w0 = pair[0][0]
                nc.sync.dma_start(
                    out=out[w0 * WIN_NODES:(w0 + nw) * WIN_NODES, :]
                    .rearrange("(w p) c -> p w c", w=nw),
                    in_=o_t[:],
                )

            cur = emit_load(groups[0])
            emit_spath(cur)
            emit_logits(cur)
            for gi in range(len(groups)):
                nxt = emit_load(groups[gi + 1]) if gi + 1 < len(groups) \
                    else None
                emit_exp_wq_mm(cur)
                if nxt is not None:
                    emit_spath(nxt)
                    emit_logits(nxt)
                emit_epilogue(cur)
                cur = nxt

    nc.finalize()
    return nc


def _host_arrays(query, key, attn_kernel, targets):
    perms, rels, T, n_slots = preprocess(
        targets, N_NODES, N_CORES, WIN_NODES
    )
    # widest 2-window group determines the resident W tile width
    Tg = [int(T[i]) + (int(T[i + 1]) if i + 1 < len(T) else 0)
          for i in range(0, len(T), 2)]
    Tgmax = max(Tg)
    wrow_1 = np.ascontiguousarray(attn_kernel.T).reshape(-1)  # [h*8+c] = A[c,h]
    wrow = np.tile(wrow_1, (128, Tgmax)).astype(np.float32)
    iota16 = np.tile(
        np.arange(WIN_NODES, dtype=np.int16), (128, Tgmax)
    )
    in_maps = []
    for c in range(N_CORES):
        qkc = np.empty((n_slots, 2 * HC), dtype=np.float32)
        qkc[:, :HC] = query[perms[c]]
        qkc[:, HC:] = key[perms[c]]
        in_maps.append({
            "qk": qkc,
            "rel": rels[c],
            "wrow": wrow,
            "iota16": iota16,
        })
    return in_maps, T, n_slots


TRACE = False          # set by test harness to capture an NTFF profile
TRACE_CORES = None
LAST_RESULTS = None    # BassKernelResults of the most recent run


def kernel(query, key, attn_kernel, targets):
    global LAST_RESULTS
    query = np.asarray(query, dtype=np.float32)
    key = np.asarray(key, dtype=np.float32)
    attn_kernel = np.asarray(attn_kernel, dtype=np.float32)
    targets = np.asarray(targets, dtype=np.int32)

    _ensure_imports()
    from concourse.bass_utils import run_bass_kernel_spmd

    in_maps, T, n_slots = _host_arrays(query, key, attn_kernel, targets)
    n_win = len(T)
    out_rows = n_win * WIN_NODES
    nc = build_nc(T, n_slots, out_rows)
    res = run_bass_kernel_spmd(
        nc, in_maps, list(range(N_CORES)),
        trace=TRACE, trace_cores=TRACE_CORES,
    )
    LAST_RESULTS = res
    shards = [res.results[c]["out"][:NODES_PER_CORE] for c in range(N_CORES)]
    out = np.concatenate(shards, axis=0).astype(np.float32)

    deg = np.bincount(targets, minlength=N_NODES)
    out[deg == 0] = 0.0
    return out
